# revision 1
# baseline (speedup 1.0000x reference)
"""IoU loss kernel for Trainium2, data-parallel over 8 NeuronCores.

Math (per box, columns = x-center, y-center, half-size s):
    w = relu(min(x+s, x'+s') - max(x-s, x'-s'))
      = relu((s+s') - max(|x-x'|, |s-s'|))          # S - max identity
    h likewise with y.
    overlap = w*h
    union   = 4s^2 + 4s'^2 - overlap = 2(S^2 + D^2) - overlap,
              S = s+s', D = s-s'
    iou     = overlap / (union + 1e-7)
    loss    = -sum(log(iou + 1e-7));  iou_sum = sum(iou)

Engine split per 128x1024-box tile:
  DVE   : dx, dy, S, D (fp32 strided reads -> fp16), abs_max fusions,
          subs, relus (tensor_scalar, 4x), overlap, union,
          tensor_tensor_reduce for iou (+ per-tile iou partial sum)
  ACT   : squares via Square(sqrt2 * x), 1/(u+eps) via Exp(-Ln(u+eps)),
          final Ln(iou+eps) with accum_out giving the loss partial sum.
          All functions live in the natural_log_exp_and_others table set.
  Host  : final [128, 2T] x 8 cores partial-sum reduction in float64.
"""

import numpy as np

import concourse.bass as bass
import concourse.mybir as mybir
from concourse import tile
from concourse.bass_utils import run_bass_kernel_spmd

N = 8388608
NCORES = 8
NS = N // NCORES  # 1048576 boxes per core
P = 128
W = 1024          # boxes per partition per tile
T = NS // (P * W)  # 8 tiles per core
EPS = 1e-7
RT2 = 1.4142135623730951

F32 = mybir.dt.float32
F16 = mybir.dt.float16
Op = mybir.AluOpType
Act = mybir.ActivationFunctionType


def _build(T_: int = T, W_: int = W, compile_passes: bool = True) -> bass.Bass:
    from concourse import bacc

    ns = P * W_ * T_
    nc = bacc.Bacc()
    outs_d = nc.dram_tensor("outputs", [ns, 3], F32, kind="ExternalInput")
    tars_d = nc.dram_tensor("targets", [ns, 3], F32, kind="ExternalInput")
    acc_d = nc.dram_tensor("acc", [P, 2 * T_], F32, kind="ExternalOutput")

    outs_v = outs_d[:, :].rearrange("(t p w) c -> t p (w c)", t=T_, p=P, w=W_)
    tars_v = tars_d[:, :].rearrange("(t p w) c -> t p (w c)", t=T_, p=P, w=W_)
    T, W = T_, W_

    from concourse.tile_rust import add_dep_helper

    with tile.TileContext(nc) as tc:
        with tc.tile_pool(name="main", bufs=2) as pool:
            accs = pool.tile([P, 2 * T], F32, tag="accs", bufs=1)
            eps_t = pool.tile([P, 1], F32, tag="eps", bufs=1)
            nc.vector.memset(eps_t[:, :], EPS)
            last_ttr = None
            RAWBUFS = 4
            ttr_hist: list = []
            dmaO_hist: list = []
            dmaT_hist: list = []
            for t in range(T):
                rawO = pool.tile([P, 3 * W], F32, tag="rawO", bufs=RAWBUFS)
                rawT = pool.tile([P, 3 * W], F32, tag="rawT", bufs=RAWBUFS)
                if t >= RAWBUFS:
                    # DMA instructions have a single sync-wait slot, but a
                    # slot-recycling load needs a WAR wait (slot readers, DVE
                    # sem), a WAW wait, and a lane-reuse wait. With bufs=4 and
                    # 2 DMAs/tile the recycled slot's DMA sits exactly 8 DMAs
                    # back, so WAW and lane-reuse share one semaphore; a
                    # Drain (large wait budget) on the SP sequencer absorbs
                    # all conditions onto the SP-seq clock, leaving the big
                    # loads wait-free.
                    dr = nc.sync.drain(fusable=False)
                    add_dep_helper(dr.ins, ttr_hist[t - RAWBUFS].ins, sync=True,
                                   reason="absorb DVE WAR tick")
                    add_dep_helper(dr.ins, dmaO_hist[t - RAWBUFS].ins, sync=True,
                                   reason="absorb old rawO DMA lane")
                    add_dep_helper(dr.ins, dmaT_hist[t - RAWBUFS].ins, sync=True,
                                   reason="absorb old rawT DMA lane")
                dmaO_hist.append(nc.sync.dma_start(out=rawO[:, :], in_=outs_v[t]))
                dmaT_hist.append(nc.sync.dma_start(out=rawT[:, :], in_=tars_v[t]))
                # The TT ISA struct has a single sync-wait slot, but dx below
                # depends on BOTH input DMAs. Absorb rawT's semaphore with a
                # tiny copy so dx only needs the rawO wait.
                dummy = pool.tile([P, 1], F32, tag="dummy")
                nc.vector.tensor_copy(dummy[:, :], rawT[:, 0:1])
                o3 = rawO.rearrange("p (w c) -> p w c", c=3)
                t3 = rawT.rearrange("p (w c) -> p w c", c=3)
                x1, y1, s1 = o3[:, :, 0], o3[:, :, 1], o3[:, :, 2]
                x2, y2, s2 = t3[:, :, 0], t3[:, :, 1], t3[:, :, 2]

                dx = pool.tile([P, W], F16, tag="dx")
                nc.vector.tensor_tensor(dx[:, :], x1, x2, Op.subtract)
                dy = pool.tile([P, W], F16, tag="dy")
                nc.vector.tensor_tensor(dy[:, :], y1, y2, Op.subtract)
                S = pool.tile([P, W], F16, tag="S")
                nc.vector.tensor_tensor(S[:, :], s1, s2, Op.add)
                D = pool.tile([P, W], F16, tag="D")
                nc.vector.tensor_tensor(D[:, :], s1, s2, Op.subtract)

                # |dx|, |dy|, |D| on the scalar engine (abs_max is CoreSim-only)
                adx = pool.tile([P, W], F16, tag="adx")
                nc.scalar.activation(adx[:, :], dx[:, :], Act.Abs)
                ady = pool.tile([P, W], F16, tag="ady")
                nc.scalar.activation(ady[:, :], dy[:, :], Act.Abs)
                aD = pool.tile([P, W], F16, tag="aD")
                nc.scalar.activation(aD[:, :], D[:, :], Act.Abs)

                mw = pool.tile([P, W], F16, tag="mw")
                nc.vector.tensor_tensor(mw[:, :], adx[:, :], aD[:, :], Op.max)
                mh = pool.tile([P, W], F16, tag="mh")
                nc.vector.tensor_tensor(mh[:, :], ady[:, :], aD[:, :], Op.max)

                wr = pool.tile([P, W], F16, tag="wr")
                nc.vector.tensor_sub(wr[:, :], S[:, :], mw[:, :])
                hr = pool.tile([P, W], F16, tag="hr")
                nc.vector.tensor_sub(hr[:, :], S[:, :], mh[:, :])

                w_ = pool.tile([P, W], F16, tag="w_")
                nc.vector.tensor_scalar_max(w_[:, :], wr[:, :], 0.0)
                h_ = pool.tile([P, W], F16, tag="h_")
                nc.vector.tensor_scalar_max(h_[:, :], hr[:, :], 0.0)

                ov = pool.tile([P, W], F16, tag="ov")
                nc.vector.tensor_mul(ov[:, :], w_[:, :], h_[:, :])

                # 2*S^2 and 2*D^2 on the scalar engine: Square(sqrt2 * x)
                qS = pool.tile([P, W], F16, tag="qS")
                nc.scalar.activation(qS[:, :], S[:, :], Act.Square, scale=RT2)
                qD = pool.tile([P, W], F16, tag="qD")
                nc.scalar.activation(qD[:, :], D[:, :], Act.Square, scale=RT2)
                qs = pool.tile([P, W], F16, tag="qs")
                nc.vector.tensor_add(qs[:, :], qS[:, :], qD[:, :])

                ue = pool.tile([P, W], F16, tag="ue")
                nc.vector.tensor_sub(ue[:, :], qs[:, :], ov[:, :])

                # r = 1/(ue + eps) = exp(-ln(ue + eps)); fp32 (can reach 1e7)
                lnu = pool.tile([P, W], F32, tag="lnu")
                nc.scalar.activation(lnu[:, :], ue[:, :], Act.Ln, bias=eps_t[:, 0:1])
                r = pool.tile([P, W], F32, tag="r")
                nc.scalar.activation(r[:, :], lnu[:, :], Act.Exp, scale=-1.0)

                # iou = overlap * r, with running per-partition sum into accs[:, t]
                iou = pool.tile([P, W], F16, tag="iou")
                nc.vector.tensor_mul(iou[:, :], ov[:, :], r[:, :])
                last_ttr = nc.vector.tensor_reduce(
                    accs[:, t : t + 1], iou[:, :], mybir.AxisListType.X, Op.add
                )

                # loss partial: sum of Ln(iou + eps) via activation accumulate
                li = pool.tile([P, W], F32, tag="li")
                last_act = nc.scalar.activation(
                    li[:, :],
                    iou[:, :],
                    Act.Ln,
                    bias=eps_t[:, 0:1],
                    accum_out=accs[:, T + t : T + t + 1],
                )
                ttr_hist.append(last_ttr)

            # acc store would need waits on both the DVE (iou accums) and ACT
            # (loss accums) sems; absorb both on an SP drain first.
            dr = nc.sync.drain(fusable=False)
            add_dep_helper(dr.ins, last_ttr.ins, sync=True,
                           reason="absorb DVE accum tick before acc store")
            add_dep_helper(dr.ins, last_act.ins, sync=True,
                           reason="absorb ACT accum tick before acc store")
            nc.sync.dma_start(out=acc_d[:, :], in_=accs[:, :])

    if compile_passes:
        # Bacc.compile runs generate_event_semaphores (splits multi-wait
        # instructions to satisfy the 1-wait-per-instruction HW limit),
        # extended-inst lowering, and ACT table loads.
        nc.compile()
    return nc


_NC_CACHE: list[bass.Bass] = []


def _get_nc() -> bass.Bass:
    if not _NC_CACHE:
        _NC_CACHE.append(_build())
    return _NC_CACHE[0]


def _run(inputs: dict, trace: bool = False, trace_kwargs: dict | None = None):
    outputs = np.ascontiguousarray(np.asarray(inputs["outputs"], dtype=np.float32))
    targets = np.ascontiguousarray(np.asarray(inputs["targets"], dtype=np.float32))
    assert outputs.shape == (N, 3) and targets.shape == (N, 3)

    nc = _get_nc()
    in_maps = [
        {
            "outputs": outputs[c * NS : (c + 1) * NS],
            "targets": targets[c * NS : (c + 1) * NS],
        }
        for c in range(NCORES)
    ]
    kw = {}
    if trace:
        kw["trace"] = True
        if trace_kwargs:
            kw["trace_kwargs"] = trace_kwargs
    res = run_bass_kernel_spmd(nc, in_maps, list(range(NCORES)), **kw)

    iou_sum = 0.0
    loss = 0.0
    for c in range(NCORES):
        acc = np.asarray(res.results[c]["acc"], dtype=np.float64)
        iou_sum += acc[:, :T].sum()
        loss += acc[:, T:].sum()
    loss = -loss
    return (np.float32(loss), np.float32(iou_sum)), res


def kernel(**inputs) -> tuple:
    (loss, iou_sum), _ = _run(inputs)
    return (loss, iou_sum)



# revision 27
# speedup vs baseline: 1.3154x; 1.3154x over previous
"""IoU loss kernel for Trainium2, data-parallel over 8 NeuronCores.

Math (per box, columns = x-center, y-center, half-size s):
    w = relu(min(x+s, x'+s') - max(x-s, x'-s'))
      = relu((s+s') - max(|x-x'|, |s-s'|))          # S - max identity
    h likewise with y.
    overlap = w*h
    union   = 4s^2 + 4s'^2 - overlap = 2(S^2 + D^2) - overlap,
              S = s+s', D = s-s'
    iou     = overlap / (union + 1e-7)
    loss    = -sum(log(iou + 1e-7));  iou_sum = sum(iou)

The DMA stream (two fp32 loads per tile, 24 MiB/core total) is the
roofline: ~76us at the 360 GB/s per-core HBM rate. Everything else is
scheduled under its shadow:

  DVE  (~7.5us/KTile): dx, dy, S, D (fp32 strided reads -> fp16, 1x
         mode), mw/mh maxes (fp16, 2x mode), relus (tensor_scalar, 4x
         mode), r = 1/(u+eps) via reciprocal_approx_fast (~18 bits).
  ACT  (~7.6us/KTile): |dx|, |dy|, |D| (Abs), 2S^2/2D^2 (Square with
         scale=sqrt2), Ln(iou+eps) accum -> loss partial, Copy(iou)
         accum -> iou partial. All funcs live in the `natural_log`
         table set -> ONE table load total (Exp would force a 1.3us
         table swap per use - hence reciprocal on DVE instead of
         Exp(-Ln(u))).
  Pool (~5.1us/KTile): wr/hr = S - m, q12 = 2S^2+2D^2+eps, ov = w*h,
         u' = q12 - ov, iou = ov*r (scalar_tensor_tensor on the
         otherwise idle GPSIMD engine).
  PE   : drains that absorb the multi-condition raw-slot-recycle waits
         (WAR + WAW + DMA lane) so the loads stay wait-free; a drain on
         the SP queue itself would stall the DMA stream ~1.7us/tile.

The per-segment work is software-pipelined with a one-segment skew
(li/ic two segments) so no engine's in-order queue ever head-of-line
blocks on the tail of the previous segment's dependency chain; the
trailing segments shrink (512/256/128/128) to cut the drain-out tail.
Host: final [128, 2*NSEG] x 8 cores partial-sum reduction in float64.
"""

import numpy as np

import concourse.bass as bass
import concourse.mybir as mybir
from concourse import tile
from concourse.bass_utils import run_bass_kernel_spmd

N = 8388608
NCORES = 8
NS = N // NCORES  # 1048576 boxes per core
P = 128
W = 1024          # boxes per partition per full tile
T = NS // (P * W)  # 8 full-tile units per core
EPS = 1e-7
RT2 = 1.4142135623730951

F32 = mybir.dt.float32
F16 = mybir.dt.float16
Op = mybir.AluOpType
Act = mybir.ActivationFunctionType


def _build(T_: int = T, W_: int = W, compile_passes: bool = True) -> bass.Bass:
    from concourse import bacc
    from concourse.tile_rust import add_dep_helper

    # Tile widths: the DMA stream is gapless, so span = stream + drain-out
    # tail. Progressively smaller trailing tiles shrink both the engine
    # queue backlog when the last load lands and the final serial chain.
    # The last three tiles use a dedicated small raw-slot tag so their
    # loads never wait on big-slot recycling.
    segs = ([W_] * (T_ - 1)
            + [W_ // 2, W_ // 4, W_ // 8, W_ // 8])
    assert sum(segs) == T_ * W_
    NSEG = len(segs)
    SMALLW = W_ // 2
    MICROW = W_ // 8
    NBIGPS = T_ - 1  # 1024-wide segs feed the PE/PSUM iou reduction

    ns = P * W_ * T_
    nc = bacc.Bacc()
    outs_d = nc.dram_tensor("outputs", [ns, 3], F32, kind="ExternalInput")
    tars_d = nc.dram_tensor("targets", [ns, 3], F32, kind="ExternalInput")
    acc_d = nc.dram_tensor("acc", [P, 2 * NSEG], F32, kind="ExternalOutput")
    iouv_d = nc.dram_tensor("iouv", [1, 1024], F32, kind="ExternalOutput")

    offs = [0]
    for w in segs:
        offs.append(offs[-1] + w)

    def seg_view(dram, s):
        b0 = P * offs[s]
        return dram[b0 : b0 + P * segs[s], :].rearrange(
            "(p w) c -> p (w c)", p=P, w=segs[s]
        )

    RAWBUFS = 3

    with tile.TileContext(nc) as tc:
        with (tc.tile_pool(name="main", bufs=2) as pool,
              tc.tile_pool(name="psum", bufs=1,
                           space=bass.MemorySpace.PSUM) as psum):
            accs = pool.tile([P, 2 * NSEG], F32, tag="accs", bufs=1)
            # big segs' iou partials live in PSUM, so their accs columns in
            # the second half are never written; zero them for the store
            nc.vector.memset(accs[:, :], 0.0)
            eps_t = pool.tile([P, 1], F32, tag="eps", bufs=1)
            nc.vector.memset(eps_t[:, :], EPS)
            ones = pool.tile([P, 1], F16, tag="ones", bufs=1)
            nc.vector.memset(ones[:, :], 1.0)
            # PSUM accumulators for the iou partition-sums: group A covers
            # segs 0..GSPLIT-1, group B the rest; each seg contributes two
            # half-width matmuls (PSUM bank = 512 fp32 columns).
            psA1 = psum.tile([1, 512], F32, tag="psA1", bufs=1)
            psA2 = psum.tile([1, 512], F32, tag="psA2", bufs=1)

            lastrd: list = []
            dmaO_h: list = []
            dmaT_h: list = []
            big_idx: list = []
            C: list = []

            def front(t):
                w = segs[t]
                if w <= MICROW:
                    rawO = pool.tile([P, 3 * MICROW], F32, tag="rawOm", bufs=3)
                    rawT = pool.tile([P, 3 * MICROW], F32, tag="rawTm", bufs=3)
                    recycle = None
                elif w <= SMALLW:
                    rawO = pool.tile([P, 3 * SMALLW], F32, tag="rawOs", bufs=3)
                    rawT = pool.tile([P, 3 * SMALLW], F32, tag="rawTs", bufs=3)
                    recycle = None
                else:
                    rawO = pool.tile([P, 3 * W_], F32, tag="rawO", bufs=RAWBUFS)
                    rawT = pool.tile([P, 3 * W_], F32, tag="rawT", bufs=RAWBUFS)
                    nbig = len(big_idx)
                    recycle = big_idx[nbig - RAWBUFS] if nbig >= RAWBUFS else None
                    big_idx.append(t)
                deps = []
                if recycle is not None:
                    dr = nc.tensor.drain(fusable=False)
                    add_dep_helper(dr.ins, lastrd[recycle].ins, sync=True,
                                   reason="absorb DVE WAR tick")
                    add_dep_helper(dr.ins, dmaO_h[recycle].ins, sync=True,
                                   reason="absorb old rawO DMA lane")
                    add_dep_helper(dr.ins, dmaT_h[recycle].ins, sync=True,
                                   reason="absorb old rawT DMA lane")
                    deps = [dr]
                dmaO = nc.sync.dma_start(out=rawO[:, : 3 * w], in_=seg_view(outs_d, t))
                dmaT = nc.sync.dma_start(out=rawT[:, : 3 * w], in_=seg_view(tars_d, t))
                for d in deps:
                    add_dep_helper(dmaO.ins, d.ins, sync=True,
                                   reason="slot guarded by PE drain")
                    add_dep_helper(dmaT.ins, d.ins, sync=True,
                                   reason="slot guarded by PE drain")
                dmaO_h.append(dmaO)
                dmaT_h.append(dmaT)

                # dx below depends on BOTH input DMAs but has one sync-wait
                # slot; absorb rawT's semaphore with a tiny copy.
                dummy = pool.tile([P, 1], F32, tag="dummy")
                nc.vector.tensor_copy(dummy[:, :], rawT[:, 0:1])
                o3 = rawO[:, : 3 * w].rearrange("p (w c) -> p w c", c=3)
                t3 = rawT[:, : 3 * w].rearrange("p (w c) -> p w c", c=3)
                x1, y1, s1 = o3[:, :, 0], o3[:, :, 1], o3[:, :, 2]
                x2, y2, s2 = t3[:, :, 0], t3[:, :, 1], t3[:, :, 2]

                c = {"t": t, "w": w}
                # tags whose only readers run in the same pipeline iteration
                # as the writer get a single buffer (the next generation's
                # WAR lands a full iteration later).
                for nm in ("dx", "dy", "D", "mw", "mh"):
                    c[nm] = pool.tile([P, W_], F16, tag=nm, bufs=1,
                                      name=f"{nm}_{t}")
                for nm in ("S", "adx", "ady", "aD", "qS", "qD", "wr", "hr",
                           "rh", "q12", "ue16", "ov", "iou", "sc"):
                    c[nm] = pool.tile([P, W_], F16, tag=nm, name=f"{nm}_{t}")
                for nm in ("ue32", "r"):
                    c[nm] = pool.tile([P, W_], F32, tag=nm, name=f"{nm}_{t}")

                # DVE: dx first so ACT's Abs chain starts as early as
                # possible (ACT otherwise idles ~0.9us/seg waiting for it)
                nc.vector.tensor_tensor(c["dx"][:, :w], x1, x2, Op.subtract)
                c["ins"] = (x1, y1, s1, x2, y2, s2)
                C.append(c)

            def front2(c):
                # DVE: remaining strided fp32 input ops (1x mode)
                x1, y1, s1, x2, y2, s2 = c["ins"]
                w = c["w"]
                nc.vector.tensor_tensor(c["dy"][:, :w], y1, y2, Op.subtract)
                nc.vector.tensor_tensor(c["S"][:, :w], s1, s2, Op.add)
                lastrd.append(
                    nc.vector.tensor_tensor(c["D"][:, :w], s1, s2, Op.subtract))

                # ACT: abs values and scaled squares (dtype/stride-blind)
                nc.scalar.activation(c["adx"][:, :w], c["dx"][:, :w], Act.Abs)
                nc.scalar.activation(c["ady"][:, :w], c["dy"][:, :w], Act.Abs)
                nc.scalar.activation(c["aD"][:, :w], c["D"][:, :w], Act.Abs)
                nc.scalar.activation(c["qS"][:, :w], c["S"][:, :w], Act.Square,
                                     scale=RT2)
                nc.scalar.activation(c["qD"][:, :w], c["D"][:, :w], Act.Square,
                                     scale=RT2)

            def maxes(c):  # DVE: thresholds, w-extent (relu in place),
                # union precursor. The h-extent runs on Pool/ACT instead.
                w = c["w"]
                nc.vector.tensor_tensor(c["mw"][:, :w], c["adx"][:, :w],
                                        c["aD"][:, :w], Op.max)
                nc.vector.tensor_tensor(c["mh"][:, :w], c["ady"][:, :w],
                                        c["aD"][:, :w], Op.max)
                nc.vector.tensor_sub(c["wr"][:, :w], c["S"][:, :w], c["mw"][:, :w])
                nc.vector.tensor_scalar_max(c["wr"][:, :w], c["wr"][:, :w], 0.0)
                nc.vector.tensor_add(c["q12"][:, :w], c["qS"][:, :w], c["qD"][:, :w])

            def hrst(c):  # Pool: hr = S - mh (Q7 ucode exists for TT only)
                w = c["w"]
                nc.gpsimd.tensor_tensor(c["hr"][:, :w], c["S"][:, :w],
                                        c["mh"][:, :w], Op.subtract)

            def rhst(c):  # ACT: relu of the h-extent
                w = c["w"]
                nc.scalar.activation(c["rh"][:, :w], c["hr"][:, :w], Act.Relu)

            def recip(c):  # DVE: r = 1/(u+eps), ~18 bits, plenty vs 2e-2
                w = c["w"]
                nc.vector.reciprocal_approx_fast(c["r"][:, :w], c["ue32"][:, :w])

            def unionst(c):  # Pool: ov = w*h, ue16 = q12 - ov
                w = c["w"]
                nc.gpsimd.tensor_tensor(c["ov"][:, :w], c["wr"][:, :w],
                                        c["rh"][:, :w], Op.mult)
                nc.gpsimd.tensor_tensor(c["ue16"][:, :w], c["q12"][:, :w],
                                        c["ov"][:, :w], Op.subtract)



            def ioust_dve(c):
                # DVE: 3/8 of iou = ov * r (1x-mode op, so DVE takes the
                # smaller share; the Q7 two-input floor caps Pool). The tiny
                # trailing segs compute the whole product here; their iou
                # partial is accumulated by an ACT Copy in accum().
                t, w = c["t"], c["w"]
                if t >= NBIGPS:
                    nc.vector.tensor_mul(c["iou"][:, :w], c["ov"][:, :w],
                                         c["r"][:, :w])
                    return
                h = (3 * w) // 8
                nc.vector.tensor_mul(c["iou"][:, :h], c["ov"][:, :h],
                                     c["r"][:, :h])

            def ioust_pool(c):  # Pool: remaining 5/8 of iou = ov * r
                w, h = c["w"], (3 * c["w"]) // 8
                if c["t"] >= NBIGPS:
                    return
                nc.gpsimd.tensor_tensor(c["iou"][:, h:w], c["ov"][:, h:w],
                                        c["r"][:, h:w], Op.mult)

            def iou_psum(c):  # PE: per-seg partition-sums into PSUM banks
                t, w, h = c["t"], c["w"], c["w"] // 2
                if t >= NBIGPS:
                    return
                first = t == 0
                last = t == NBIGPS - 1
                nc.tensor.matmul(psA1[:, :h], ones[:, :], c["iou"][:, :h],
                                 start=first, stop=last)
                nc.tensor.matmul(psA2[:, :h], ones[:, :],
                                 c["iou"][:, h : 2 * h],
                                 start=first, stop=last)

            def ue32st(c):  # ACT: fp32 upcast with the eps floor.
                # Exact math guarantees u >= (a1+a2)/2 = q12/2, so the f16
                # rounding of q12 - ov can never go below ~-1 ulp of q12/2;
                # Relu(ue16 + eps) therefore stays in (0, inf) and feeds the
                # fp32-only reciprocal bit-trick safely.
                w = c["w"]
                nc.scalar.activation(c["ue32"][:, :w], c["ue16"][:, :w],
                                     Act.Relu, bias=eps_t[:, 0:1])

            def accum(c):  # ACT: loss partial rides the Ln accumulator;
                # trailing segs also get their iou partial via a Copy accum
                # (the big segs' iou partials ride the PE/PSUM reduction).
                t, w = c["t"], c["w"]
                nc.scalar.activation(
                    c["sc"][:, :w], c["iou"][:, :w], Act.Ln,
                    bias=eps_t[:, 0:1],
                    accum_out=accs[:, t : t + 1],
                )
                if t >= NBIGPS:
                    nc.scalar.activation(
                        c["sc"][:, :w], c["iou"][:, :w], Act.Copy,
                        accum_out=accs[:, NSEG + t : NSEG + t + 1],
                    )

            # Five-stage software pipeline: every cross-engine dependency
            # lands at least one full segment before its consumer, so no
            # engine queue ever blocks mid-iteration. Per-iteration queue
            # orders (front-loaded ready work first):
            #   DVE : mw,mh(k-1) | r(k-4) | dx..D(k) | wr,relus,q12(k-1)
            #   ACT : li,ic(k-5) | ue32(k-3) | abs,squares(k)
            #   Pool: ov,ue16(k-2) | hr(k-1) | iou(k-4)
            def ps_extract(bank, col, n):
                # ACT copies one PSUM bank into a small staging tile which
                # streams straight out to DRAM.
                pscp = pool.tile([1, 512], F32, tag="pscp", name=f"pscp_{col}")
                nc.scalar.copy(pscp[:, :n], bank[:, :n])
                nc.sync.dma_start(out=iouv_d[:, col : col + n],
                                  in_=pscp[:, :n])

            for k in range(NSEG + 5):
                if 4 <= k <= NSEG + 3:
                    accum(C[k - 4])
                    iou_psum(C[k - 4])
                if 2 <= k <= NSEG + 1:
                    unionst(C[k - 2])
                if k < NSEG:
                    front(k)
                if 3 <= k <= NSEG + 2:
                    recip(C[k - 3])
                    ioust_dve(C[k - 3])
                    ioust_pool(C[k - 3])
                if 1 <= k <= NSEG:
                    maxes(C[k - 1])
                    hrst(C[k - 1])
                if k < NSEG:
                    front2(C[k])
                if 2 <= k <= NSEG + 1:
                    ue32st(C[k - 2])
                if 1 <= k <= NSEG:
                    rhst(C[k - 1])
                if k == NBIGPS + 4:
                    # the PSUM group closed at iter NBIGPS+3; drain its banks
                    # while the trailing small tiles stream
                    ps_extract(psA1, 0, 512)
                    ps_extract(psA2, 512, 512)

            # accs is written only by ACT accumulators; the store needs just
            # the ACT sem tick of the final Copy, which fits the single DMA
            # wait slot.
            nc.sync.dma_start(out=acc_d[:, :], in_=accs[:, :])

    if compile_passes:
        # Bacc.compile runs generate_event_semaphores (splits multi-wait
        # instructions to satisfy the 1-wait-per-instruction HW limit),
        # extended-inst lowering, and ACT table loads.
        nc.compile()
    return nc


_NC_CACHE: list[bass.Bass] = []


def _get_nc() -> bass.Bass:
    if not _NC_CACHE:
        _NC_CACHE.append(_build())
    return _NC_CACHE[0]


def _run(inputs: dict, trace: bool = False, trace_kwargs: dict | None = None):
    outputs = np.ascontiguousarray(np.asarray(inputs["outputs"], dtype=np.float32))
    targets = np.ascontiguousarray(np.asarray(inputs["targets"], dtype=np.float32))
    assert outputs.shape == (N, 3) and targets.shape == (N, 3)

    nc = _get_nc()
    in_maps = [
        {
            "outputs": outputs[c * NS : (c + 1) * NS],
            "targets": targets[c * NS : (c + 1) * NS],
        }
        for c in range(NCORES)
    ]
    kw = {}
    if trace:
        kw["trace"] = True
        if trace_kwargs:
            kw["trace_kwargs"] = trace_kwargs
    res = run_bass_kernel_spmd(nc, in_maps, list(range(NCORES)), **kw)

    iou_sum = 0.0
    loss = 0.0
    for c in range(NCORES):
        acc = np.asarray(res.results[c]["acc"], dtype=np.float64)
        half = acc.shape[1] // 2
        loss += acc[:, :half].sum()
        iou_sum += acc[:, half:].sum()
        iou_sum += np.asarray(res.results[c]["iouv"], dtype=np.float64).sum()
    loss = -loss
    return (np.float32(loss), np.float32(iou_sum)), res


def kernel(**inputs) -> tuple:
    (loss, iou_sum), _ = _run(inputs)
    return (loss, iou_sum)
